# revision 30
# baseline (speedup 1.0000x reference)
"""Trainium2 kernel for nn_LoRALinear (moe_routing).

Math: reference computes out = x @ W.T + einsum('bri,bro->bo', a, b) with
a = A_table[dom].reshape(B,R,IN), b = B_table[dom].reshape(B,R,OUT).
The einsum contracts i over `a` alone, so the LoRA term collapses to a
per-domain table:
    L[d, o] = sum_r (sum_i A_table[d].reshape(R,IN)[r,i]) * B_table[d].reshape(R,OUT)[r,o]
    out = x @ W.T + L[domain_id]

On device: dense x @ W.T on the PE (K = 1024 as 8 chunks of 128); the
LoRA rows L[domain_id] are gathered on the HOST into a per-core stream
`lg` and added during the psum->SBUF eviction on the Vector engine
(GPSIMD cannot read PSUM), so the PE runs only the 16 dense matmuls per
128-row tile. Output is written bf16 (rel-err budget is 2e-2) and
widened to f32 on the host.

Sharding: data-parallel over batch across 8 cores; W replicated.

Schedule notes (from perfetto/NTFF analysis):
- Only sync (SP) and scalar (ACT) have hardware DGE rings; descriptors
  drain FIFO per ring, so each ring's DMAs are emitted in need order.
  The block-0 prologue is early-DMA-rate bound (~180-250 GB/s), so W
  chunks are split across both rings, interleaved with the x pairs.
- Per-partition DMA lines below 2KB run the engines at ~half rate;
  every transfer here keeps lines >= 2KB.
- 24 short warmup matmuls bridge the framework preamble (~7us) to the
  first data arrival: the PE clock ramps over ~3-5us from the first
  matmul and the ramp RESTARTS after an idle gap, so the warmup must
  hand off to real matmuls without a bubble.
- Block 0's 4 m-tiles are k-interleaved across all 8 psum banks so the
  PE saturates as soon as chunk-pair 0 lands; blocks 1-3 run sequential
  per-m-tile chains over the same 8-bank rotation.
- Out-DMAs ride the sync ring; the next block's x/lg DMAs are emitted
  BEFORE the previous block's finishes so their issue instructions do
  not queue behind the out-DMAs' semaphore waits.
"""

import functools

import numpy as np

import concourse.mybir as mybir
import concourse.tile as tile
from concourse import bacc, bass_utils

B, D, R, ND = 16384, 1024, 8, 64
N_CORES = 8
BS = B // N_CORES            # 2048 batch rows per core
NKW = 8                      # K chunks of 128 for the dense W part
MB = 512                     # batch rows per x block
NMB = BS // MB               # 4 blocks
OH = 512                     # psum free dim (one bank)


@functools.lru_cache(maxsize=1)
def _build():
    nc = bacc.Bacc(None, target_bir_lowering=False, debug=False)
    bf16 = mybir.dt.bfloat16
    add = mybir.AluOpType.add
    # chunk-major x: xa[p, (mb*NKW + k)*MB + j] = x[mb*MB+j, k*128+p]
    xa = nc.dram_tensor("xa", [128, NMB * NKW * MB], bf16, kind="ExternalInput")
    wa = nc.dram_tensor("wa", [NKW * 128, D], bf16, kind="ExternalInput")
    # host-gathered LoRA rows, m-tile major per block:
    # lg[p, (mb*4 + mt)*D + o] = L[dom[mb*MB + mt*128 + p], o]
    lg = nc.dram_tensor("lg", [128, NMB * 4 * D], bf16, kind="ExternalInput")
    out = nc.dram_tensor("out", [BS, D], bf16, kind="ExternalOutput")

    with tile.TileContext(nc) as tc:
        with (
            tc.tile_pool(name="w", bufs=1) as wpool,
            tc.tile_pool(name="x", bufs=2) as xpool,
            tc.tile_pool(name="l", bufs=2) as lpool,
            tc.tile_pool(name="o", bufs=4) as opool,
            tc.tile_pool(name="ps", bufs=8, space="PSUM") as pspool,
        ):
            # Dummy matmuls on a zero scratch tile start the HAM clock
            # ramp while the first chunk DMAs are still in flight.
            scratch = wpool.tile([128, OH], bf16, tag="scratch")
            nc.vector.memset(scratch[:], 0.0)
            dps = pspool.tile([128, OH], mybir.dt.float32, tag="ps")
            NWARM = 24
            for i in range(NWARM):
                nc.tensor.matmul(
                    dps[:, 0:128],
                    scratch[:, 0:128],
                    scratch[:, 0:128],
                    start=(i == 0),
                    stop=(i == NWARM - 1),
                )

            # Balance the two hardware rings during the prologue: x chunk
            # pairs + odd W chunks on sync, even W chunks on scalar, all
            # in need order so each sweep k waits only for its own pair.
            xts = {}
            xt0 = xpool.tile([128, NKW * MB], bf16, tag="x")
            wts = []
            for k in range(NKW):
                wt = wpool.tile([128, D], bf16, tag=f"w{k}")
                wts.append(wt)

            def wdma(eng, k):
                eng.dma_start(wts[k][:], wa[k * 128 : (k + 1) * 128, :])

            wdma(nc.scalar, 0)
            for p in range(4):
                a = 2 * p
                nc.sync.dma_start(
                    xt0[:, a * MB : (a + 2) * MB], xa[:, a * MB : (a + 2) * MB]
                )
                wdma(nc.sync, 2 * p + 1)
                if p < 3:
                    wdma(nc.scalar, 2 * p + 2)
            xts[0] = xt0

            # lg rides the scalar ring behind W (need-ordered there); the
            # sync ring carries only the x stream plus out-DMAs so neither
            # ring starves the prologue-critical W chunks.
            lts = {}
            lt0 = lpool.tile([128, 4 * D], bf16, tag="l")
            nc.scalar.dma_start(lt0[:], lg[:, 0 : 4 * D])
            lts[0] = lt0

            def xsl(xt, k, mt):
                return xt[:, k * MB + mt * 128 : k * MB + (mt + 1) * 128]

            def finish(mt, ps0, ps1, mb, lt):
                """psum + L[dom] -> bf16 out tile, then DMA out."""
                ot = opool.tile([128, D], bf16, tag="ot")
                nc.vector.scalar_tensor_tensor(
                    ot[:, 0:OH], ps0[:], 0.0, lt[:, mt * D : mt * D + OH], add, add
                )
                nc.vector.scalar_tensor_tensor(
                    ot[:, OH:D], ps1[:], 0.0, lt[:, mt * D + OH : (mt + 1) * D], add, add
                )
                m0 = mb * MB + mt * 128
                nc.sync.dma_start(out[m0 : m0 + 128, :], ot[:])

            def prefetch(mb):
                """Emit block mb's x/lg DMAs. Emitted BEFORE the previous
                block's finishes so the issue doesn't queue behind the
                out-DMAs' semaphore waits on the sync ring."""
                xtn = xpool.tile([128, NKW * MB], bf16, tag="x")
                nc.sync.dma_start(
                    xtn[:], xa[:, mb * NKW * MB : (mb + 1) * NKW * MB]
                )
                xts[mb] = xtn
                ltn = lpool.tile([128, 4 * D], bf16, tag="l")
                nc.scalar.dma_start(ltn[:], lg[:, mb * 4 * D : (mb + 1) * 4 * D])
                lts[mb] = ltn

            # Block 0: k-interleaved across all 4 m-tiles x 2 halves so
            # each arriving chunk pair immediately feeds 8 matmuls.
            pss = {}
            for g in range(8):
                psg = pspool.tile([128, OH], mybir.dt.float32, tag="ps")
                pss[g] = psg
            for k in range(NKW):
                for g in range(8):
                    mt, h = divmod(g, 2)
                    nc.tensor.matmul(
                        pss[g][:],
                        xsl(xt0, k, mt),
                        wts[k][:, h * OH : (h + 1) * OH],
                        start=(k == 0),
                        stop=(k == NKW - 1),
                    )
            prefetch(1)
            for mt in range(4):
                finish(mt, pss[2 * mt], pss[2 * mt + 1], 0, lt0)

            # Blocks 1-3: sequential per-m-tile chains; pool rotation
            # keeps the PE fed while finishes drain.
            for mb in range(1, NMB):
                xt = xts[mb]
                for mt in range(4):
                    if mt == 0 and mb + 1 < NMB:
                        prefetch(mb + 1)
                    ps0 = pspool.tile([128, OH], mybir.dt.float32, tag="ps")
                    ps1 = pspool.tile([128, OH], mybir.dt.float32, tag="ps")
                    for k in range(NKW):
                        nc.tensor.matmul(
                            ps0[:],
                            xsl(xt, k, mt),
                            wts[k][:, 0:OH],
                            start=(k == 0),
                            stop=(k == NKW - 1),
                        )
                    if not (mb == NMB - 1 and mt == 3):
                        for k in range(NKW):
                            nc.tensor.matmul(
                                ps1[:],
                                xsl(xt, k, mt),
                                wts[k][:, OH : 2 * OH],
                                start=(k == 0),
                                stop=(k == NKW - 1),
                            )
                        finish(mt, ps0, ps1, mb, lts[mb])
                    else:
                        # Final m-tile: drain half 0 while half 1 still
                        # matmuls; each half goes out as its own DMA.
                        lt = lts[mb]
                        ot = opool.tile([128, D], bf16, tag="ot")
                        m0 = mb * MB + mt * 128
                        nc.vector.scalar_tensor_tensor(
                            ot[:, 0:OH], ps0[:], 0.0,
                            lt[:, mt * D : mt * D + OH], add, add,
                        )
                        nc.sync.dma_start(
                            out[m0 : m0 + 128, 0:OH], ot[:, 0:OH]
                        )
                        for k in range(NKW):
                            nc.tensor.matmul(
                                ps1[:],
                                xsl(xt, k, mt),
                                wts[k][:, OH : 2 * OH],
                                start=(k == 0),
                                stop=(k == NKW - 1),
                            )
                        nc.vector.scalar_tensor_tensor(
                            ot[:, OH:D], ps1[:], 0.0,
                            lt[:, mt * D + OH : (mt + 1) * D], add, add,
                        )
                        nc.sync.dma_start(
                            out[m0 : m0 + 128, OH:D], ot[:, OH:D]
                        )

    nc.compile()
    return nc


def _prepare(x, W, A_table, B_table, domain_id):
    import ml_dtypes

    bf16 = np.dtype(ml_dtypes.bfloat16)
    x = np.asarray(x, dtype=np.float32)
    W = np.asarray(W, dtype=np.float32)
    A = np.asarray(A_table, dtype=np.float64)
    Bt = np.asarray(B_table, dtype=np.float64)
    dom = np.asarray(domain_id).astype(np.int64)

    sA = A.reshape(ND, R, D).sum(axis=2)                        # [ND, R]
    L = np.einsum("dr,dro->do", sA, Bt.reshape(ND, R, D))       # [ND, D]
    Lb = L.astype(np.float32).astype(bf16)                      # [ND, D]
    Lg = Lb[dom]                                                # [B, D]

    wa = np.ascontiguousarray(W.T.astype(bf16))                 # [D, D]
    xT = np.ascontiguousarray(x.T).astype(bf16)                 # [D, B]

    in_maps = []
    for c in range(N_CORES):
        sl = slice(c * BS, (c + 1) * BS)
        # chunk-major: xa[p, (mb*NKW + k)*MB + j] = xT[k*128+p, c*BS + mb*MB + j]
        xa_c = np.ascontiguousarray(
            xT[:, sl].reshape(NKW, 128, NMB, MB).transpose(1, 2, 0, 3)
        ).reshape(128, NMB * NKW * MB)
        # m-tile major: lg[p, (mb*4+mt)*D + o] = Lg[c*BS + mb*MB + mt*128 + p, o]
        lg_c = np.ascontiguousarray(
            Lg[sl].reshape(NMB, 4, 128, D).transpose(2, 0, 1, 3)
        ).reshape(128, NMB * 4 * D)
        in_maps.append({"xa": xa_c, "wa": wa, "lg": lg_c})
    return in_maps


def kernel(x, W, A_table, B_table, domain_id, _trace=False):
    in_maps = _prepare(x, W, A_table, B_table, domain_id)
    nc = _build()
    res = bass_utils.run_bass_kernel_spmd(
        nc, in_maps, core_ids=list(range(N_CORES)), trace=_trace
    )
    out = np.concatenate(
        [res.results[c]["out"].astype(np.float32) for c in range(N_CORES)], axis=0
    )
    if _trace:
        kernel.last_results = res
    return out
